# revision 4
# baseline (speedup 1.0000x reference)
"""Trainium2 Bass kernel for KMeans assignment (argmin over 8192 centroids).

Problem: x [32768, 1024] f32, centroids [1024, 8192] f32 ->
         argmin_k ||x_n - c_k||^2  as int32 [32768].

Math: argmin_k ||x_n - c_k||^2 == argmax_k (x.c_k - 0.5*||c_k||^2).

Device strategy (data-parallel over N, 8 cores x 4096 rows):
  Inputs are quantized to a 0.5 grid whose scaled values are EXACT in
  fp8-e4m3:
    X = round(2x) in [-15,15],  C = round(2c) in [-15,15]
  so the device dot product sum(X*C) = 4*(x_q . c_q) is an exact integer
  in f32 PSUM.  Each PSUM span of 2048 centroids (4 banks) is opened by
  four K=3 fp16 matmuls (lhsT = ones[3,128]) injecting
      bias_hi + bias_r + j*2^-11
  where bias_hi + bias_r = round(4*bias_true) (exact int, |.| < 4096, both
  rows exact fp16) and j in [0,2048) is the in-span column index
  (j*2^-11 in [0,1), exact fp16).  Then 4 fp8 DoubleRow matmuls per
  512-chunk (contraction 256 each) accumulate the dot at 2x PE
  throughput.  The final PSUM value is a key
        key = (sum XC + round(4*bias)) + j * 2^-11
  integer part = 4*score_q, fraction = the in-span index, exact in f32
  (|key| < 2^12, lsb 2^-11 -> 23 bits).  One DVE max8 per span reads PSUM
  directly: no bias tensor_tensor, no max_index pass, no PSUM->SBUF
  copies.  Host decodes 32 candidates/row from the keys and re-scores
  them exactly (fp32 then fp64 on the top-4) before the final argmax.
"""
import os
import numpy as np

# ---- problem constants (hardcoded per harness contract) ----
N_FULL, D, K = 32768, 1024, 8192
N_CORES = 8
NC = N_FULL // N_CORES          # 4096 rows per core
RT = 128                        # rows per tile
NT = NC // RT                   # 32 row-tiles per core
CHUNK = 512                     # one PSUM bank / one matmul moving extent
SPAN = 2048                     # one DVE max8 extent (4 banks)
KC = K // CHUNK                 # 16 chunks
NSPAN = K // SPAN               # 4 spans
SC = SPAN // CHUNK              # 4 chunks per span
DP = 4                          # DoubleRow pairs: 1024 = 4 * 256
IOTA = 1.0 / SPAN               # 2^-11 in-span index step

_compiled = {}


def _build(nt_count=NT):
    """Build + compile the per-core Bass program."""
    from contextlib import ExitStack
    import concourse.bacc as bacc
    import concourse.mybir as mybir
    import concourse.tile as tile

    f32 = mybir.dt.float32
    f16 = mybir.dt.float16
    fp8 = mybir.dt.float8e4
    DR = mybir.MatmulPerfMode.DoubleRow

    nc = bacc.Bacc("TRN2", target_bir_lowering=False, debug=False)

    ncols = nt_count * RT
    xt_d = nc.dram_tensor("xt", [D, ncols], fp8, kind="ExternalInput").ap()
    c_d = nc.dram_tensor("cent", [D, K], fp8, kind="ExternalInput").ap()
    b_d = nc.dram_tensor("brow", [3, K], f16, kind="ExternalInput").ap()
    keys_d = nc.dram_tensor("keys", [nt_count, 128, NSPAN * 8], f32,
                            kind="ExternalOutput").ap()

    with tile.TileContext(nc) as tc:
        with ExitStack() as ctx:
            const_pool = ctx.enter_context(tc.tile_pool(name="const", bufs=1))
            mv_pool = ctx.enter_context(tc.tile_pool(name="mv", bufs=2))
            ps_pool = ctx.enter_context(
                tc.tile_pool(name="psum", bufs=2, space="PSUM"))

            ones_sb = const_pool.tile([3, RT], f16, name="ones_sb")
            nc.vector.memset(ones_sb[:], 1.0)
            brow_sb = const_pool.tile([3, K], f16, name="brow_sb")
            nc.sync.dma_start(brow_sb[:], b_d[:])

            # x^T and centroids, laid out for DoubleRow: plane i of pair d2
            # holds contraction rows [d2*256 + i*128, +128)
            xt_sb = const_pool.tile([128, DP, 2, ncols], fp8, name="xt_sb")
            c_sb = const_pool.tile([128, DP, 2, K], fp8, name="c_sb")
            for d2 in range(DP):
                for i in range(2):
                    r0 = d2 * 256 + i * 128
                    nc.sync.dma_start(xt_sb[:, d2, i, :],
                                      xt_d[r0:r0 + 128, :])
                    nc.sync.dma_start(c_sb[:, d2, i, :],
                                      c_d[r0:r0 + 128, :])

            for t in range(nt_count):
                mv = mv_pool.tile([128, NSPAN * 8], f32, name="mv")
                for sp in range(NSPAN):
                    ps = ps_pool.tile([128, SPAN], f32, name="ps")
                    k0 = sp * SPAN
                    # open all 4 bank-groups of the span with bias + iota
                    for q in range(SC):
                        nc.tensor.matmul(
                            ps[:, q * CHUNK:(q + 1) * CHUNK], ones_sb[:],
                            brow_sb[:, k0 + q * CHUNK:k0 + (q + 1) * CHUNK],
                            start=True, stop=False)
                    # accumulate the dot; lhsT constant across the 4 chunks
                    for d2 in range(DP):
                        for q in range(SC):
                            nc.tensor.matmul(
                                ps[:, q * CHUNK:(q + 1) * CHUNK],
                                xt_sb[:, d2, :, t * RT:(t + 1) * RT],
                                c_sb[:, d2, :,
                                     k0 + q * CHUNK:k0 + (q + 1) * CHUNK],
                                start=False, stop=(d2 == DP - 1),
                                perf_mode=DR)
                    nc.vector.max(mv[:, sp * 8:(sp + 1) * 8], ps[:])
                nc.sync.dma_start(keys_d[t], mv[:])
    nc.compile()
    return nc


def _get_nc(nt_count=NT):
    if nt_count not in _compiled:
        _compiled[nt_count] = _build(nt_count)
    return _compiled[nt_count]


def _prep_inputs(x, centroids):
    """Quantize to the exact-fp8 grid and build device input arrays."""
    import ml_dtypes

    X = np.clip(np.rint(x * 2.0), -15, 15).astype(np.float32)       # [N, D]
    C = np.clip(np.rint(centroids * 2.0), -15, 15).astype(np.float32)
    xt_dev = np.ascontiguousarray(X.T).astype(ml_dtypes.float8_e4m3)
    c_dev = np.ascontiguousarray(C).astype(ml_dtypes.float8_e4m3)

    c64 = centroids.astype(np.float64)
    bias_true = -0.5 * np.einsum("dk,dk->k", c64, c64)              # [K]
    bias_dev = np.rint(4.0 * bias_true)
    assert np.abs(bias_dev).max() < 4096, "bias exceeds exact range"
    bias_r = bias_dev - 2.0 * np.rint(bias_dev / 2.0)               # {-1,0,1}
    bias_hi = bias_dev - bias_r                                     # even int
    brow = np.zeros((3, K), np.float16)
    brow[0] = bias_hi
    brow[1] = bias_r
    brow[2] = (np.arange(K) % SPAN) * IOTA
    return xt_dev, c_dev, brow, bias_true


def kernel(x: np.ndarray, centroids: np.ndarray) -> np.ndarray:
    from concourse.bass_utils import run_bass_kernel_spmd

    x = np.asarray(x, dtype=np.float32)
    centroids = np.asarray(centroids, dtype=np.float32)
    nc = _get_nc(NT)

    xt_dev, c_dev, brow, bias_true = _prep_inputs(x, centroids)

    in_maps = []
    for c in range(N_CORES):
        in_maps.append({
            "xt": np.ascontiguousarray(xt_dev[:, c * NC:(c + 1) * NC]),
            "cent": c_dev,
            "brow": brow,
        })

    res = run_bass_kernel_spmd(nc, in_maps, core_ids=list(range(N_CORES)))

    out = np.empty(N_FULL, dtype=np.int32)
    for c in range(N_CORES):
        keys = res.results[c]["keys"].astype(np.float64)    # [NT,128,32]
        cand = _decode(keys)                                # [NC, 32]
        out[c * NC:(c + 1) * NC] = _refine(
            x[c * NC:(c + 1) * NC], centroids, bias_true, cand)
    return out


def _decode(keys):
    """keys [nt, 128, NSPAN*8] -> global candidate indices [nt*128, NSPAN*8]."""
    keys = keys.reshape(-1, 128, NSPAN, 8)
    frac = keys - np.floor(keys)                   # j * 2^-11 in [0, 1)
    j = np.rint(frac * SPAN).astype(np.int64)      # in-span index
    gk = j + (np.arange(NSPAN) * SPAN)[None, None, :, None]
    return gk.reshape(-1, NSPAN * 8)


def _refine(xs, centroids, bias_true, cand, top=4):
    """Re-score all candidates in fp32, then the top-`top` exactly in fp64.
    Ties broken to the smallest global index (argmin semantics)."""
    n, ncand = cand.shape
    out = np.empty(n, dtype=np.int32)
    cT32 = np.ascontiguousarray(centroids.T)                # [K, D] f32
    cT64 = cT32.astype(np.float64)
    b32 = bias_true.astype(np.float32)
    bs = 2048
    for s in range(0, n, bs):
        e = min(s + bs, n)
        cb = cand[s:e]                                      # [b, 32]
        cc = cT32[cb]                                       # [b, 32, D] f32
        sc = np.matmul(cc, xs[s:e, :, None])[..., 0]        # [b, 32]
        sc += b32[cb]
        part = np.argpartition(-sc, top - 1, axis=1)[:, :top]
        cb4 = np.take_along_axis(cb, part, axis=1)          # [b, top]
        cc4 = cT64[cb4]
        sc4 = np.matmul(cc4, xs[s:e, :, None].astype(np.float64))[..., 0]
        sc4 += bias_true[cb4]
        best = sc4.max(axis=1, keepdims=True)
        big = np.where(sc4 >= best, cb4, np.iinfo(np.int64).max)
        out[s:e] = big.min(axis=1).astype(np.int32)
    return out
